# revision 5
# baseline (speedup 1.0000x reference)
"""DTW distance kernel for Trainium2 (8 NeuronCores, SPMD data-parallel over batch).

Per core: NB=16 batch elements.
Phase 1 (cost matrix): cost[b][i,j] = ||x[b,:,i] - y[b,:,j]||_2 via bf16 PE matmuls
  d2 = (-2x)^T y + x2[i]*1 + 1*y2[j]  (3 accumulated matmuls per 128-row chunk),
  then ACT sqrt -> bf16 staging -> SBUF reshuffle DMAs into the DP layout.
Phase 2 (DP): dtw wavefront. 8 column-strips x 16 batches = 128 partitions.
  Strip s lags strip s-1 by L steps. Per step t (strip s handles row i = t - L*s):
    m    = min(R[t-1][:,1:W+1], R[t-1][:,0:W])            (DVE)
    R[t][:,1:W+1] = scan(min(m, state) + cost; state0 = R[t][:,0])  (DVE)
    boundary: R[t+L][16:128, 0] <- R[t][0:112, W]         (tiny SBUF DMA, partition shift)
  Inactive strip lanes are kept at ~1e30 ("BIG") so garbage never contaminates
  valid lanes (costdp head pads are BIG; R slots init to BIG).
"""
import sys
import numpy as np

sys.path.insert(0, "/opt/trn_rl_repo")

import concourse.bass as bass  # noqa: E402
import concourse.bacc as bacc  # noqa: E402
import concourse.mybir as mybir  # noqa: E402
import concourse.tile as tile  # noqa: E402

NCORES = 8
B_FULL, F_FULL, T_FULL = 128, 128, 512
BIG = 1.0e30


def build_dtw(nb, F, T, S, W, L, nslot=32):
    """Build the per-core SPMD Bass graph. Partition p = s*nb + b."""
    assert S * W == T
    P = S * nb
    assert P <= 128
    NC = (T + 127) // 128
    CM = T // NC  # chunk rows (i per matmul chunk)
    assert CM * NC == T
    TS = L * (S - 1) + T  # DP steps
    assert nslot > L + 2
    f32, bf16 = mybir.dt.float32, mybir.dt.bfloat16
    mn, ad, mu = mybir.AluOpType.min, mybir.AluOpType.add, mybir.AluOpType.mult
    AF = mybir.ActivationFunctionType

    nc = bacc.Bacc(None, target_bir_lowering=False, debug=False, num_swdge_queues=4)
    x = nc.declare_dram_parameter("x", [nb, F, T], f32, isOutput=False)
    y = nc.declare_dram_parameter("y", [nb, F, T], f32, isOutput=False)
    out = nc.declare_dram_parameter("out", [nb, 1], f32, isOutput=True)

    with tile.TileContext(nc) as tc:
        with (
            tc.tile_pool(name="persist", bufs=1) as pp,
            tc.tile_pool(name="xin", bufs=3) as xin,
            tc.tile_pool(name="sq", bufs=3) as sqp,
            tc.tile_pool(name="stg", bufs=3) as stgp,
            tc.tile_pool(name="m", bufs=4) as mp,
            tc.tile_pool(name="ps_d2", bufs=2, space="PSUM") as psd,
            tc.tile_pool(name="ps_rows", bufs=2, space="PSUM") as psr,
        ):
            # ---- constants / persistent state ----
            ones_col = pp.tile([F, 1], bf16, tag="ones_col")
            nc.gpsimd.memset(ones_col[:], 1.0)
            ones_row = pp.tile([1, T], bf16, tag="ones_row")
            nc.gpsimd.memset(ones_row[:], 1.0)
            quarter_row = pp.tile([1, T], bf16, tag="quarter_row")
            nc.gpsimd.memset(quarter_row[:], 0.25)
            costdp = pp.tile([P, TS, W], bf16, tag="costdp")
            R = [pp.tile([P, W + 1], f32, tag=f"R{k}", name=f"R{k}") for k in range(nslot)]
            for k in range(nslot):
                nc.gpsimd.memset(R[k][:], BIG)
            # strip-0, t=0 scan initial = 0 (r[0][-1] == 0 for the DP corner)
            nc.gpsimd.memset(R[0][0:nb, 0:1], 0.0)
            # BIG source for costdp head pads (strips s>0, steps t < L*s)
            max_pad = L * (S - 1) * W
            if max_pad > 0:
                bigt = pp.tile([nb, max_pad], bf16, tag="bigt")
                nc.gpsimd.memset(bigt[:], BIG)
                for s in range(1, S):
                    nc.sync.dma_start(
                        costdp[s * nb:(s + 1) * nb, 0:L * s, :],
                        bigt[:, 0:L * s * W],
                    )
                for s in range(S - 1):
                    tail = L * (S - 1 - s)
                    nc.sync.dma_start(
                        costdp[s * nb:(s + 1) * nb, L * s + T:TS, :],
                        bigt[:, 0:tail * W],
                    )

            xm2 = [pp.tile([F, T], bf16, tag=f"xm2_{b}", name=f"xm2_{b}") for b in range(nb)]
            yb = [pp.tile([F, T], bf16, tag=f"yb_{b}", name=f"yb_{b}") for b in range(nb)]
            sqr = [pp.tile([1, 2 * T], bf16, tag=f"sqr_{b}", name=f"sqr_{b}") for b in range(nb)]

            # ---- stage A: load, cast, squares, row sums ----
            for b in range(nb):
                tx = xin.tile([F, T], f32, tag="tx")
                ty = xin.tile([F, T], f32, tag="ty")
                nc.sync.dma_start(tx[:], x[b])
                nc.sync.dma_start(ty[:], y[b])
                nc.scalar.activation(xm2[b][:], tx[:], AF.Copy, scale=-2.0)
                nc.scalar.activation(yb[b][:], ty[:], AF.Copy, scale=1.0)
                xsq = sqp.tile([F, T], bf16, tag="xsq")
                ysq = sqp.tile([F, T], bf16, tag="ysq")
                nc.vector.tensor_tensor(xsq[:], xm2[b][:], xm2[b][:], op=mu)  # 4x^2
                nc.vector.tensor_tensor(ysq[:], yb[b][:], yb[b][:], op=mu)    # y^2
                pr = psr.tile([1, 2 * T], f32, tag="pr")
                nc.tensor.matmul(pr[0:1, 0:T], ones_col[:], xsq[:], start=True, stop=True)
                nc.tensor.matmul(pr[0:1, T:2 * T], ones_col[:], ysq[:], start=True, stop=True)
                # sqr row: [4*x2 | y2] (x2 quarter-scaled later via quarter_row)
                nc.scalar.activation(sqr[b][:], pr[:], AF.Copy, scale=1.0)

            # ---- interleaved: stage B (per chunk) + DP steps ----
            def dp_step(t):
                slot, pslot = t % nslot, (t - 1) % nslot
                m = mp.tile([P, W], f32, tag="m")
                nc.vector.tensor_tensor(
                    m[:], R[pslot][:, 1:W + 1], R[pslot][:, 0:W], op=mn)
                nc.vector.tensor_tensor_scan(
                    R[slot][:, 1:W + 1], m[:], costdp[:, t, :],
                    R[slot][:, 0:1], op0=mn, op1=ad)
                if t == 0:
                    # retire the one-time 0.0 initial so slot reuse sees BIG
                    nc.gpsimd.memset(R[0][0:nb, 0:1], BIG)
                if t + L < TS and S > 1:
                    nc.gpsimd.dma_start(
                        R[(t + L) % nslot][nb:P, 0:1],
                        R[slot][0:P - nb, W:W + 1])

            t_next = 0
            for c in range(NC):
                for b in range(nb):
                    ps = psd.tile([CM, T], f32, tag="ps")
                    nc.tensor.matmul(
                        ps[:], xm2[b][:, c * CM:(c + 1) * CM], yb[b][:],
                        start=True, stop=False)
                    nc.tensor.matmul(
                        ps[:], sqr[b][0:1, c * CM:(c + 1) * CM], quarter_row[:],
                        start=False, stop=False)
                    nc.tensor.matmul(
                        ps[:], ones_row[0:1, c * CM:(c + 1) * CM],
                        sqr[b][0:1, T:2 * T], start=False, stop=True)
                    stg = stgp.tile([CM, T], bf16, tag="stg")
                    nc.scalar.activation(stg[:], ps[:], AF.Sqrt)
                    for s in range(S):
                        q = s * nb + b
                        nc.gpsimd.dma_start(
                            costdp[q:q + 1, L * s + c * CM: L * s + (c + 1) * CM, :],
                            stg[:, s * W:(s + 1) * W])
                t_hi = TS if c == NC - 1 else (c + 1) * CM
                for t in range(t_next, t_hi):
                    dp_step(t)
                t_next = t_hi

            # ---- extract answers: strip S-1, row T-1, col W ----
            slotf = (TS - 1) % nslot
            nc.sync.dma_start(out[:], R[slotf][(S - 1) * nb:P, W:W + 1])

    nc.compile()
    return nc


_cache = {}


def _get_nc():
    key = "full"
    if key not in _cache:
        _cache[key] = build_dtw(
            nb=B_FULL // NCORES, F=F_FULL, T=T_FULL, S=8, W=64, L=16)
    return _cache[key]


def kernel(x, y):
    from concourse.bass_utils import run_bass_kernel_spmd

    x = np.ascontiguousarray(x, dtype=np.float32)
    y = np.ascontiguousarray(y, dtype=np.float32)
    nb = B_FULL // NCORES
    nc = _get_nc()
    in_maps = [
        {"x": x[c * nb:(c + 1) * nb], "y": y[c * nb:(c + 1) * nb]}
        for c in range(NCORES)
    ]
    res = run_bass_kernel_spmd(nc, in_maps, list(range(NCORES)))
    outs = [res.results[c]["out"].reshape(nb) for c in range(NCORES)]
    return np.concatenate(outs).astype(np.float32)
